# revision 59
# baseline (speedup 1.0000x reference)
"""Supervised-contrastive loss (balanced softmax variant) on 8 Trainium2 cores.

Transposed/class-sorted formulation. Rows are sorted by target class and
split 1024 per core. Columns (all 8192 features + 1000 centers, merged and
class-sorted) are permuted per core:
  - the core's "positive window" (all columns whose class appears among its
    rows, <= 1280 of 9216) forms 10 masked j-tiles;
  - the remaining columns are re-sorted by their per-class count cc so that
    PAIRS of j-tiles carry identical per-partition bias vectors (each
    (pair, partition) slot holds two columns of equal cc; odd cc-buckets are
    evened out with dummy zero columns whose exactly-known contribution
    a*e^-10 is subtracted on the host). A pair's two tiles share one
    [128, 2048] ACT instruction, amortizing the per-instruction overhead.

Processing slots (41 = 31 pairs + 10 masked singles, masked every 4th):
    PE : d[j, i] = A_j . f_i                    fp8(e4m3) matmul, N=512
    ACT: E'[j, i] = exp(10*d + (ln a_j - 10))   a_j = 1/cc rides in the
         per-PARTITION activation bias -- no bias matmul at all
    DVE: Acc[s%2] += E'          fp16 tensor_tensor add (2x_1p mode)
    DVE (masked slots): MAcc += (tcol_j == trow_i) * E'

Final partition-reduction on the PE (ones-column matmuls) produces
S = colsum(Acc0+Acc1) and P = colsum(MAcc) as [1, 1024] rows; one 8 KB
DMA returns them. The host finishes in f64:
    S'_i  = S_i - dummy_contam + (P_i - Eii)/n_i
    mlp_i = 10*(f_i.M[t_i] - r2_i)/n_i - 10 - log(S'_i)
    loss  = -mean(mlp_i)
where Eii = exp(10*r2q_i - 10) removes the self column exactly (r2q is the
self dot in the same fp8 values the PE saw), and M[c] is the class sum of
fp16 features + center.
"""

import sys
from contextlib import ExitStack

import numpy as np
import ml_dtypes

sys.path.insert(0, "/opt/trn_rl_repo")

import concourse.bass as bass  # noqa: E402
import concourse.mybir as mybir  # noqa: E402
import concourse.tile as tile  # noqa: E402
from concourse import bacc  # noqa: E402
from concourse.bass_utils import run_bass_kernel_spmd  # noqa: E402

P = 128
BL = 1024          # rows per core
MW = 10            # masked-window j-tiles (1280 cols >= max window)
NPAIR = 31         # paired j-tiles among the rest columns
NT = MW + 2 * NPAIR            # 72 j-tiles of 128 columns
JP = NT * P                    # 9216 padded columns
NSLOT = MW + NPAIR             # 41 processing slots
TEMP = 0.1
SHIFT = 10.0

# slot schedule: masked singles run CONSECUTIVELY (slots 5-14): inside the
# run the cheap masked matmuls hide under the masked EXPs, so the 2-deep
# PSUM ring only stalls once at the run exit instead of after every masked
# slot. Their DVE work is narrowed to a 320-column i-window (below) and
# deep et buffering absorbs the temporary DVE backlog.
MSLOTS = list(range(5, 5 + MW))
# masked tile m's positives live in a narrow sorted-row window: rows and
# window columns are both class-sorted, so row index ~ 0.8 * column index.
# The true need for these inputs is [102.4m - 2, 102.4m + 205); +-~50 margin.
WWIN = 320
WOFF = [min(max(round(102.4 * m) - 48, 0), BL - WWIN) for m in range(MW)]
KINDS = ["M" if s in MSLOTS else "P" for s in range(NSLOT)]

# fTq DMA chunk sizes in j-tiles: small first so the PE can start early
WCHUNKS = [2] * 6 + [4] * 3 + [6] * 8
assert sum(WCHUNKS) == NT
FCHUNK = 4         # featq DMA chunks

F8NP = ml_dtypes.float8_e4m3
F8 = mybir.dt.float8e4
F16 = mybir.dt.float16
F32 = mybir.dt.float32
AF = mybir.ActivationFunctionType
ALU = mybir.AluOpType


def build_nc() -> bass.Bass:
    """One-core program; run SPMD on 8 cores with per-core inputs."""
    nc = bacc.Bacc(None)
    # per-chunk weight/feature params: each is a contiguous row-major block
    # in DRAM so its DMA is one linear burst per partition run
    featq_ds = [
        nc.declare_dram_parameter(f"featq{c}", [P, BL // FCHUNK], F8,
                                  isOutput=False)
        for c in range(FCHUNK)
    ]
    fTq_ds = [
        nc.declare_dram_parameter(f"fTq{ci}", [P, w * P], F8, isOutput=False)
        for ci, w in enumerate(WCHUNKS)
    ]
    bias_d = nc.declare_dram_parameter("bias", [P, NSLOT], F32, isOutput=False)
    tIrow_d = nc.declare_dram_parameter("tIrow", [1, BL], F16, isOutput=False)
    tcol_d = nc.declare_dram_parameter("tcol", [P, MW], F16, isOutput=False)
    out_d = nc.declare_dram_parameter("out", [1, 2 * BL], F32, isOutput=True)

    with tile.TileContext(nc) as tc, ExitStack() as ctx:
        const = ctx.enter_context(tc.tile_pool(name="const", bufs=1))
        epool2 = ctx.enter_context(tc.tile_pool(name="epool2", bufs=4))
        epool1 = ctx.enter_context(tc.tile_pool(name="epool1", bufs=6))
        mpool = ctx.enter_context(tc.tile_pool(name="mpool", bufs=2))
        psum2 = ctx.enter_context(
            tc.tile_pool(name="psum2", bufs=2, space=bass.MemorySpace.PSUM)
        )

        # warm the ACT exp table while DMAs are in flight
        warm = const.tile([P, 1], F32)
        nc.vector.memset(warm[:], 0.0)
        nc.scalar.activation(warm[:], warm[:], AF.Exp, bias=warm[:], scale=1.0)

        # -- input DMAs; issue split between the Sync and GpSimd sequencers
        # (each dma_start costs ~600ns of serial issue time per sequencer).
        # Priority order: what the first loop iterations need comes first.
        featq = const.tile([P, BL], F8)
        fTqs = []

        def ftq_dma(eng, ci):
            off = sum(WCHUNKS[:ci])
            w = WCHUNKS[ci]
            ft = const.tile([P, w * P], F8, tag=f"fTq{ci}")
            eng.dma_start(ft[:], fTq_ds[ci][:])
            fTqs.append((off, w, ft))

        FS = BL // FCHUNK
        for c in range(FCHUNK):
            nc.sync.dma_start(featq[:, c * FS:(c + 1) * FS], featq_ds[c][:])
            ftq_dma(nc.sync, c)
        for ci in range(4, len(WCHUNKS)):
            ftq_dma(nc.sync, ci)

        bias = const.tile([P, NSLOT], F32)
        nc.gpsimd.dma_start(bias[:, 0:8], bias_d[:, 0:8])
        tIrow_r = const.tile([1, BL], F16)
        nc.gpsimd.dma_start(tIrow_r[:], tIrow_d[:])
        tcol = const.tile([P, MW], F16)
        nc.gpsimd.dma_start(tcol[:], tcol_d[:])
        nc.gpsimd.dma_start(bias[:, 8:NSLOT], bias_d[:, 8:NSLOT])
        tIrow = const.tile([P, BL], F16)
        nc.gpsimd.partition_broadcast(tIrow[:], tIrow_r[:])
        fTqs.sort(key=lambda x: x[0])

        ones = const.tile([P, 1], F16)
        nc.vector.memset(ones[:], 1.0)
        scratch = const.tile([P, 512], F16)
        nc.vector.memset(scratch[:], 0.0)
        acc0 = const.tile([P, 2 * BL], F16)
        nc.vector.memset(acc0[:], 0.0)
        acc1 = const.tile([P, 2 * BL], F16)
        nc.vector.memset(acc1[:], 0.0)
        macc = const.tile([P, BL], F16)
        nc.vector.memset(macc[:], 0.0)
        accs = [acc0, acc1]

        outt = const.tile([1, 2 * BL], F32)

        # prologue: throwaway matmul+exp rounds rotate the PSUM ring and fill
        # the PE/ACT pipelines before the first data-dependent tile
        scrap = const.tile([1, 512], F16)
        for _ in range(2):
            prot = psum2.tile([P, 2 * BL], F32, tag="pt2")
            nc.tensor.matmul(prot[0:1, 0:512], ones[:], scratch[:],
                             start=True, stop=True)
            nc.scalar.activation(scrap[:], prot[0:1, 0:512], AF.Exp,
                                 bias=warm[0:1, :], scale=1.0)

        def jtile(t):
            for off, w, ft in fTqs:
                if off <= t < off + w:
                    return ft[:, (t - off) * P:(t - off + 1) * P]
            raise AssertionError

        t = 0          # running j-tile index (layout follows slot order)
        for s in range(NSLOT):
            if KINDS[s] == "P":
                pt = psum2.tile([P, 2 * BL], F32, tag="pt2")
                for q in range(2):
                    lhs = jtile(t + q)
                    for h in range(2):
                        nc.tensor.matmul(
                            pt[:, q * BL + h * 512:q * BL + (h + 1) * 512],
                            lhs, featq[:, h * 512:(h + 1) * 512],
                            start=True, stop=True,
                        )
                et = epool2.tile([P, 2 * BL], F16, tag="et2")
                nc.scalar.activation(
                    et[:], pt[:], AF.Exp, bias=bias[:, s:s + 1], scale=SHIFT,
                )
                if s == NSLOT - 1:
                    et_last = et    # reduced straight into sred, no DVE add
                else:
                    a = accs[s % 2]
                    nc.vector.tensor_tensor(a[:], a[:], et[:], ALU.add)
                t += 2
            else:
                m = MSLOTS.index(s)
                pt = psum2.tile([P, 2 * BL], F32, tag="pt2")
                lhs = jtile(t)
                for h in range(2):
                    nc.tensor.matmul(
                        pt[:, h * 512:(h + 1) * 512], lhs,
                        featq[:, h * 512:(h + 1) * 512],
                        start=True, stop=True,
                    )
                et = epool1.tile([P, BL], F16, tag="et1")
                nc.scalar.activation(
                    et[:], pt[:, 0:BL], AF.Exp, bias=bias[:, s:s + 1],
                    scale=SHIFT,
                )
                a = accs[s % 2]
                nc.vector.tensor_tensor(a[:, 0:BL], a[:, 0:BL], et[:], ALU.add)
                o = WOFF[m]
                mt = mpool.tile([P, WWIN], F16, tag="mt")
                nc.vector.scalar_tensor_tensor(
                    out=mt[:], in0=tIrow[:, o:o + WWIN],
                    scalar=tcol[:, m:m + 1], in1=et[:, o:o + WWIN],
                    op0=ALU.is_equal, op1=ALU.mult,
                )
                nc.vector.tensor_tensor(macc[:, o:o + WWIN],
                                        macc[:, o:o + WWIN], mt[:], ALU.add)
                t += 1
            if s == MSLOTS[-1] + 8:
                # masked accumulator is final (with slack for DVE to drain):
                # reduce + stage its output early, copy on DVE so the Scalar
                # EXP stream is not interrupted
                predt = psum2.tile([P, 2 * BL], F32, tag="pt2")
                for h in range(2):
                    nc.tensor.matmul(predt[0:1, h * 512:(h + 1) * 512], ones[:],
                                     macc[:, h * 512:(h + 1) * 512],
                                     start=True, stop=True)
                nc.vector.tensor_scalar_add(outt[:, BL:2 * BL],
                                            predt[0:1, 0:BL], 0.0)
                nc.sync.dma_start(out_d[:, BL:2 * BL], outt[:, BL:2 * BL])
        assert t == NT

        # S reduction: sred[i] = sum over partitions of all acc halves plus
        # the last pair's E' (reduced directly, skipping its DVE add).
        # acc1 goes idle first, acc0 next, et_last at EXP(40) -- emitted in
        # that order so only et_last's matmuls trail the final EXP.
        sredt = psum2.tile([P, 2 * BL], F32, tag="pt2")
        for h in range(2):
            sl = slice(h * 512, (h + 1) * 512)
            srcs = [acc0[:, h * 512:(h + 1) * 512],
                    acc0[:, BL + h * 512:BL + (h + 1) * 512],
                    acc1[:, h * 512:(h + 1) * 512],
                    acc1[:, BL + h * 512:BL + (h + 1) * 512],
                    et_last[:, h * 512:(h + 1) * 512],
                    et_last[:, BL + h * 512:BL + (h + 1) * 512]]
            for i, src in enumerate(srcs):
                nc.tensor.matmul(sredt[0:1, sl], ones[:], src,
                                 start=(i == 0), stop=(i == len(srcs) - 1))
        nc.scalar.copy(outt[:, 0:512], sredt[0:1, 0:512])
        nc.vector.tensor_scalar_add(outt[:, 512:BL], sredt[0:1, 512:BL], 0.0)
        nc.sync.dma_start(out_d[:, 0:BL], outt[:, 0:BL])

    nc.finalize()
    return nc


def prep_inputs(centers1, features, targets, n_cores):
    """Host-side sort/shard/layout prep. Returns per-core input maps and
    the per-core host epilogue data."""
    B, D = features.shape
    C = centers1.shape[0]
    J = B + C
    assert BL * n_cores == B and D == P and JP >= J

    feats16 = np.asarray(features, np.float32).astype(np.float16)
    cents16 = np.asarray(centers1, np.float32).astype(np.float16)
    targets = np.asarray(targets).astype(np.int64)

    n = np.bincount(targets, minlength=C).astype(np.int64)
    cc = n + 1

    order = np.argsort(targets, kind="stable")          # rows sorted by class
    # merged class-sorted columns: per class, feature rows then the center
    col_order = np.lexsort((
        np.concatenate([np.arange(B), np.full(C, B)]),
        np.concatenate([targets, np.arange(C)]),
    ))
    A16 = np.concatenate([feats16, cents16], axis=0)[col_order]   # [J, D]
    col_cls = np.concatenate([targets, np.arange(C)])[col_order]  # [J]

    # class sums for the numerator (f64 on the fp16 values)
    M = np.zeros((C, D))
    np.add.at(M, targets, feats16.astype(np.float64))
    M += cents16

    in_maps, host = [], []
    for k in range(n_cores):
        rids = order[k * BL:(k + 1) * BL]
        tcls = targets[rids]
        jlo = np.searchsorted(col_cls, tcls[0], "left")
        jhi = np.searchsorted(col_cls, tcls[-1], "right")
        assert jhi - jlo <= MW * P, f"core {k}: window {jhi - jlo} > {MW * P}"
        perm = np.concatenate([
            np.arange(jlo, jhi), np.arange(0, jlo), np.arange(jhi, J)
        ])
        win = perm[:MW * P]              # 10 masked tiles (window + borrow)
        restc = perm[MW * P:]            # remaining real columns

        # all positives of masked tile m must fall inside its fixed i-window
        for m in range(MW):
            wcls = col_cls[win[m * P:(m + 1) * P]]
            match = np.where(np.isin(tcls, wcls))[0]
            if len(match):
                assert WOFF[m] <= match[0] and match[-1] < WOFF[m] + WWIN, \
                    f"core {k} tile {m}: rows [{match[0]},{match[-1]}] " \
                    f"outside window {WOFF[m]}+{WWIN}"

        # pair the rest by cc: sort by cc, even out each bucket with dummies
        ccr = cc[col_cls[restc]]
        o2 = np.argsort(ccr, kind="stable")
        restc, ccr = restc[o2], ccr[o2]
        cols2, bias2, contam = [], [], 0.0   # col index (-1 = dummy), bias
        for v in np.unique(ccr):
            idxs = restc[ccr == v]
            b = np.log(1.0 / v) - SHIFT
            cols2.extend(idxs); bias2.extend([b] * len(idxs))
            if len(idxs) % 2:
                cols2.append(-1); bias2.append(b)
                contam += (1.0 / v) * np.exp(-SHIFT)
        npads = 2 * NPAIR * P - len(cols2)
        assert npads >= 0, f"core {k}: cc padding overflow ({-npads})"
        cols2.extend([-1] * npads); bias2.extend([-200.0] * npads)
        cols2 = np.array(cols2); bias2 = np.array(bias2)

        # j-tile layout in slot order + per-slot bias
        tiles, biass = [], np.empty(NSLOT)
        q = 0
        for s in range(NSLOT):
            if KINDS[s] == "P":
                sl = slice(q * 2 * P, (q + 1) * 2 * P)
                tiles.append(cols2[sl][0::2])    # tile A: even positions
                tiles.append(cols2[sl][1::2])    # tile B: odd positions
                biass_s = bias2[sl][0::2]
                q += 1
            else:
                m = MSLOTS.index(s)
                wcols = win[m * P:(m + 1) * P]
                tiles.append(wcols)
                biass_s = np.log(1.0 / cc[col_cls[wcols]]) - SHIFT
            if KINDS[s] == "P":
                # bias per partition: even/odd positions share cc by design
                bias_col = biass_s
            else:
                bias_col = biass_s
            if s == 0:
                biasm = np.empty((P, NSLOT))
            biasm[:, s] = bias_col
        assert q == NPAIR and len(tiles) == NT

        Ap = np.zeros((JP, D), np.float16)
        for ti, colidx in enumerate(tiles):
            real = colidx >= 0
            Ap[ti * P:(ti + 1) * P][real] = A16[colidx[real]]

        fq8 = feats16[rids].astype(F8NP)         # [BL, 128]
        A8 = Ap.astype(F8NP)                     # [JP, 128]

        im = {
            "bias": np.ascontiguousarray(biasm.astype(np.float32)),
            "tIrow": tcls.astype(np.float16).reshape(1, BL),
            "tcol": np.ascontiguousarray(
                col_cls[win].reshape(MW, P).T.astype(np.float16)
            ),
        }
        featq = np.ascontiguousarray(fq8.T)       # [128, BL]
        FS = BL // FCHUNK
        for c in range(FCHUNK):
            im[f"featq{c}"] = np.ascontiguousarray(featq[:, c * FS:(c + 1) * FS])
        fTq = A8.reshape(NT, P, P).transpose(2, 0, 1).reshape(P, NT * P)
        off = 0
        for ci, w in enumerate(WCHUNKS):
            im[f"fTq{ci}"] = np.ascontiguousarray(fTq[:, off * P:(off + w) * P])
            off += w
        in_maps.append(im)

        n_t = n[tcls].astype(np.float64)
        fqd = fq8.astype(np.float64)
        r2q = np.einsum("ij,ij->i", fqd, fqd)
        Eii = np.exp(SHIFT * r2q.astype(np.float32).astype(np.float64) - SHIFT)
        f16d = feats16[rids].astype(np.float64)
        r2n = np.einsum("ij,ij->i", f16d, f16d)
        numer = (SHIFT * (np.einsum("ij,ij->i", f16d, M[tcls]) - r2n)) / n_t - SHIFT
        host.append({"n_t": n_t, "Eii": Eii, "numer": numer, "contam": contam})
    return in_maps, host


_NC_CACHE = {}


def _get_nc():
    if "nc" not in _NC_CACHE:
        _NC_CACHE["nc"] = build_nc()
    return _NC_CACHE["nc"]


def run(centers1, features, targets, trace=False):
    n_cores = 8
    nc = _get_nc()
    in_maps, host = prep_inputs(centers1, features, targets, n_cores)
    res = run_bass_kernel_spmd(nc, in_maps, list(range(n_cores)), trace=trace)
    mlps = []
    for k in range(n_cores):
        out = res.results[k]["out"].astype(np.float64).reshape(-1)  # [2*BL]
        S = out[0:BL]
        Pm = out[BL:2 * BL]
        h = host[k]
        Sfix = S - h["contam"] + (Pm - h["Eii"]) / h["n_t"]
        mlps.append(h["numer"] - np.log(Sfix))
    loss = -np.mean(np.concatenate(mlps))
    return np.float32(loss), res


def kernel(centers1, features, targets):
    loss, _ = run(centers1, features, targets)
    return np.asarray(loss, dtype=np.float32)


# revision 66
# speedup vs baseline: 1.0019x; 1.0019x over previous
"""Supervised-contrastive loss (balanced softmax variant) on 8 Trainium2 cores.

Transposed/class-sorted formulation. Rows are sorted by target class and
split 1024 per core. Columns (all 8192 features + 1000 centers, merged and
class-sorted) are permuted per core:
  - the core's "positive window" (all columns whose class appears among its
    rows, <= 1280 of 9216) forms 10 masked j-tiles;
  - the remaining columns are re-sorted by their per-class count cc so that
    PAIRS of j-tiles carry identical per-partition bias vectors (each
    (pair, partition) slot holds two columns of equal cc; odd cc-buckets are
    evened out with dummy zero columns whose exactly-known contribution
    a*e^-10 is subtracted on the host). A pair's two tiles share one
    [128, 2048] ACT instruction, amortizing the per-instruction overhead.

Processing slots (41 = 31 pairs + 10 masked singles, masked every 4th):
    PE : d[j, i] = A_j . f_i                    fp8(e4m3) matmul, N=512
    ACT: E'[j, i] = exp(10*d + (ln a_j - 10))   a_j = 1/cc rides in the
         per-PARTITION activation bias -- no bias matmul at all
    DVE: Acc[s%2] += E'          fp16 tensor_tensor add (2x_1p mode)
    DVE (masked slots): MAcc += (tcol_j == trow_i) * E'

Final partition-reduction on the PE (ones-column matmuls) produces
S = colsum(Acc0+Acc1) and P = colsum(MAcc) as [1, 1024] rows; one 8 KB
DMA returns them. The host finishes in f64:
    S'_i  = S_i - dummy_contam + (P_i - Eii)/n_i
    mlp_i = 10*(f_i.M[t_i] - r2_i)/n_i - 10 - log(S'_i)
    loss  = -mean(mlp_i)
where Eii = exp(10*r2q_i - 10) removes the self column exactly (r2q is the
self dot in the same fp8 values the PE saw), and M[c] is the class sum of
fp16 features + center.
"""

import sys
from contextlib import ExitStack

import numpy as np
import ml_dtypes

sys.path.insert(0, "/opt/trn_rl_repo")

import concourse.bass as bass  # noqa: E402
import concourse.mybir as mybir  # noqa: E402
import concourse.tile as tile  # noqa: E402
from concourse import bacc  # noqa: E402
from concourse.bass_utils import run_bass_kernel_spmd  # noqa: E402

P = 128
BL = 1024          # rows per core
MW = 10            # masked-window j-tiles (1280 cols >= max window)
NSING = 4          # unpaired single j-tiles: 2 at the start, 2 at the end
NPAIR = 29         # paired j-tiles among the rest columns
NT = MW + NSING + 2 * NPAIR    # 72 j-tiles of 128 columns
JP = NT * P                    # 9216 padded columns
NSLOT = MW + NSING + NPAIR     # 43 processing slots
TEMP = 0.1
SHIFT = 10.0

# slot schedule:
#  - slots 0,1 and 41,42 are SINGLES: the first EXP waits on only 2 cold
#    matmuls, and the last slots' E' reduce straight into sred with just
#    2 trailing matmuls each;
#  - masked singles run CONSECUTIVELY (slots 5-14): inside the run the
#    cheap masked matmuls hide under the masked EXPs, so the 2-deep PSUM
#    ring only stalls once at the run exit instead of after every masked
#    slot. Their DVE work is narrowed to a 320-column i-window (below)
#    and deep et buffering absorbs the temporary DVE backlog.
MSLOTS = list(range(5, 5 + MW))
SSLOTS = [0, 1, NSLOT - 2, NSLOT - 1]
# masked tile m's positives live in a narrow sorted-row window: rows and
# window columns are both class-sorted, so row index ~ 0.8 * column index.
# The true need for these inputs is [102.4m - 2, 102.4m + 205); +-~50 margin.
WWIN = 320
WOFF = [min(max(round(102.4 * m) - 48, 0), BL - WWIN) for m in range(MW)]
KINDS = ["S" if s in SSLOTS else "M" if s in MSLOTS else "P"
         for s in range(NSLOT)]

# fTq DMA chunk sizes in j-tiles: small first so the PE can start early
WCHUNKS = [1, 1] + [2] * 5 + [4] * 3 + [6] * 8
assert sum(WCHUNKS) == NT
FCHUNK = 4         # featq DMA chunks

F8NP = ml_dtypes.float8_e4m3
F8 = mybir.dt.float8e4
F16 = mybir.dt.float16
F32 = mybir.dt.float32
AF = mybir.ActivationFunctionType
ALU = mybir.AluOpType


def build_nc() -> bass.Bass:
    """One-core program; run SPMD on 8 cores with per-core inputs."""
    nc = bacc.Bacc(None)
    # per-chunk weight/feature params: each is a contiguous row-major block
    # in DRAM so its DMA is one linear burst per partition run
    featq_ds = [
        nc.declare_dram_parameter(f"featq{c}", [P, BL // FCHUNK], F8,
                                  isOutput=False)
        for c in range(FCHUNK)
    ]
    fTq_ds = [
        nc.declare_dram_parameter(f"fTq{ci}", [P, w * P], F8, isOutput=False)
        for ci, w in enumerate(WCHUNKS)
    ]
    bias_d = nc.declare_dram_parameter("bias", [P, NSLOT], F32, isOutput=False)
    tIrow_d = nc.declare_dram_parameter("tIrow", [1, BL], F16, isOutput=False)
    tcol_d = nc.declare_dram_parameter("tcol", [P, MW], F16, isOutput=False)
    out_d = nc.declare_dram_parameter("out", [1, 2 * BL], F32, isOutput=True)

    with tile.TileContext(nc) as tc, ExitStack() as ctx:
        const = ctx.enter_context(tc.tile_pool(name="const", bufs=1))
        epool2 = ctx.enter_context(tc.tile_pool(name="epool2", bufs=4))
        epool1 = ctx.enter_context(tc.tile_pool(name="epool1", bufs=6))
        mpool = ctx.enter_context(tc.tile_pool(name="mpool", bufs=2))
        psum2 = ctx.enter_context(
            tc.tile_pool(name="psum2", bufs=2, space=bass.MemorySpace.PSUM)
        )

        # warm the ACT exp table while DMAs are in flight
        warm = const.tile([P, 1], F32)
        nc.vector.memset(warm[:], 0.0)
        nc.scalar.activation(warm[:], warm[:], AF.Exp, bias=warm[:], scale=1.0)

        # -- input DMAs; issue split between the Sync and GpSimd sequencers
        # (each dma_start costs ~600ns of serial issue time per sequencer).
        # Priority order: what the first loop iterations need comes first.
        featq = const.tile([P, BL], F8)
        fTqs = []

        def ftq_dma(eng, ci):
            off = sum(WCHUNKS[:ci])
            w = WCHUNKS[ci]
            ft = const.tile([P, w * P], F8, tag=f"fTq{ci}")
            eng.dma_start(ft[:], fTq_ds[ci][:])
            fTqs.append((off, w, ft))

        FS = BL // FCHUNK
        for c in range(FCHUNK):
            nc.sync.dma_start(featq[:, c * FS:(c + 1) * FS], featq_ds[c][:])
            ftq_dma(nc.sync, c)
        for ci in range(4, len(WCHUNKS)):
            ftq_dma(nc.sync, ci)

        bias = const.tile([P, NSLOT], F32)
        nc.gpsimd.dma_start(bias[:, 0:8], bias_d[:, 0:8])
        tIrow_r = const.tile([1, BL], F16)
        nc.gpsimd.dma_start(tIrow_r[:], tIrow_d[:])
        tcol = const.tile([P, MW], F16)
        nc.gpsimd.dma_start(tcol[:], tcol_d[:])
        nc.gpsimd.dma_start(bias[:, 8:NSLOT], bias_d[:, 8:NSLOT])
        tIrow = const.tile([P, BL], F16)
        nc.gpsimd.partition_broadcast(tIrow[:], tIrow_r[:])
        fTqs.sort(key=lambda x: x[0])

        ones = const.tile([P, 1], F16)
        nc.vector.memset(ones[:], 1.0)
        scratch = const.tile([P, 512], F16)
        nc.vector.memset(scratch[:], 0.0)
        acc0 = const.tile([P, 2 * BL], F16)
        nc.vector.memset(acc0[:], 0.0)
        acc1 = const.tile([P, 2 * BL], F16)
        nc.vector.memset(acc1[:], 0.0)
        macc = const.tile([P, BL], F16)
        nc.vector.memset(macc[:], 0.0)
        accs = [acc0, acc1]

        outt = const.tile([1, 2 * BL], F32)

        # prologue: throwaway matmul+exp rounds rotate the PSUM ring and fill
        # the PE/ACT pipelines before the first data-dependent tile
        scrap = const.tile([1, 512], F16)
        prot = psum2.tile([P, 2 * BL], F32, tag="pt2")
        nc.tensor.matmul(prot[0:1, 0:512], ones[:], scratch[:],
                         start=True, stop=True)
        nc.scalar.activation(scrap[:], prot[0:1, 0:512], AF.Exp,
                             bias=warm[0:1, :], scale=1.0)

        def jtile(t):
            for off, w, ft in fTqs:
                if off <= t < off + w:
                    return ft[:, (t - off) * P:(t - off + 1) * P]
            raise AssertionError

        t = 0          # running j-tile index (layout follows slot order)
        et_tail = []   # end-singles' E', reduced straight into sred
        for s in range(NSLOT):
            if KINDS[s] == "S":
                pt = psum2.tile([P, 2 * BL], F32, tag="pt2")
                lhs = jtile(t)
                for h in range(2):
                    nc.tensor.matmul(
                        pt[:, h * 512:(h + 1) * 512], lhs,
                        featq[:, h * 512:(h + 1) * 512],
                        start=True, stop=True,
                    )
                et = epool1.tile([P, BL], F16, tag="et1")
                nc.scalar.activation(
                    et[:], pt[:, 0:BL], AF.Exp, bias=bias[:, s:s + 1],
                    scale=SHIFT,
                )
                if s >= NSLOT - 2:
                    et_tail.append(et)
                else:
                    a = accs[s % 2]
                    nc.vector.tensor_tensor(a[:, 0:BL], a[:, 0:BL], et[:],
                                            ALU.add)
                t += 1
            elif KINDS[s] == "P":
                pt = psum2.tile([P, 2 * BL], F32, tag="pt2")
                for q in range(2):
                    lhs = jtile(t + q)
                    for h in range(2):
                        nc.tensor.matmul(
                            pt[:, q * BL + h * 512:q * BL + (h + 1) * 512],
                            lhs, featq[:, h * 512:(h + 1) * 512],
                            start=True, stop=True,
                        )
                et = epool2.tile([P, 2 * BL], F16, tag="et2")
                nc.scalar.activation(
                    et[:], pt[:], AF.Exp, bias=bias[:, s:s + 1], scale=SHIFT,
                )
                a = accs[s % 2]
                nc.vector.tensor_tensor(a[:], a[:], et[:], ALU.add)
                t += 2
            else:
                m = MSLOTS.index(s)
                pt = psum2.tile([P, 2 * BL], F32, tag="pt2")
                lhs = jtile(t)
                for h in range(2):
                    nc.tensor.matmul(
                        pt[:, h * 512:(h + 1) * 512], lhs,
                        featq[:, h * 512:(h + 1) * 512],
                        start=True, stop=True,
                    )
                et = epool1.tile([P, BL], F16, tag="et1")
                nc.scalar.activation(
                    et[:], pt[:, 0:BL], AF.Exp, bias=bias[:, s:s + 1],
                    scale=SHIFT,
                )
                a = accs[s % 2]
                nc.vector.tensor_tensor(a[:, 0:BL], a[:, 0:BL], et[:], ALU.add)
                o = WOFF[m]
                mt = mpool.tile([P, WWIN], F16, tag="mt")
                nc.vector.scalar_tensor_tensor(
                    out=mt[:], in0=tIrow[:, o:o + WWIN],
                    scalar=tcol[:, m:m + 1], in1=et[:, o:o + WWIN],
                    op0=ALU.is_equal, op1=ALU.mult,
                )
                nc.vector.tensor_tensor(macc[:, o:o + WWIN],
                                        macc[:, o:o + WWIN], mt[:], ALU.add)
                t += 1
            if s == MSLOTS[-1] + 8:
                # masked accumulator is final (with slack for DVE to drain):
                # reduce + stage its output early, copy on DVE so the Scalar
                # EXP stream is not interrupted
                predt = psum2.tile([P, 2 * BL], F32, tag="pt2")
                for h in range(2):
                    nc.tensor.matmul(predt[0:1, h * 512:(h + 1) * 512], ones[:],
                                     macc[:, h * 512:(h + 1) * 512],
                                     start=True, stop=True)
                nc.vector.tensor_scalar_add(outt[:, BL:2 * BL],
                                            predt[0:1, 0:BL], 0.0)
                nc.sync.dma_start(out_d[:, BL:2 * BL], outt[:, BL:2 * BL])
        assert t == NT

        # S reduction: sred[i] = sum over partitions of all acc halves plus
        # the two end-singles' E' (reduced directly, skipping their DVE
        # adds). acc1 goes idle first, acc0 next, the end-singles at their
        # EXPs -- emitted in that order so only the end-singles' matmuls
        # trail the final EXP.
        sredt = psum2.tile([P, 2 * BL], F32, tag="pt2")
        for h in range(2):
            sl = slice(h * 512, (h + 1) * 512)
            srcs = [acc1[:, h * 512:(h + 1) * 512],
                    acc1[:, BL + h * 512:BL + (h + 1) * 512],
                    acc0[:, h * 512:(h + 1) * 512],
                    acc0[:, BL + h * 512:BL + (h + 1) * 512],
                    et_tail[0][:, h * 512:(h + 1) * 512],
                    et_tail[1][:, h * 512:(h + 1) * 512]]
            for i, src in enumerate(srcs):
                nc.tensor.matmul(sredt[0:1, sl], ones[:], src,
                                 start=(i == 0), stop=(i == len(srcs) - 1))
        nc.scalar.copy(outt[:, 0:512], sredt[0:1, 0:512])
        nc.vector.tensor_scalar_add(outt[:, 512:BL], sredt[0:1, 512:BL], 0.0)
        nc.sync.dma_start(out_d[:, 0:BL], outt[:, 0:BL])

    nc.finalize()
    return nc


def prep_inputs(centers1, features, targets, n_cores):
    """Host-side sort/shard/layout prep. Returns per-core input maps and
    the per-core host epilogue data."""
    B, D = features.shape
    C = centers1.shape[0]
    J = B + C
    assert BL * n_cores == B and D == P and JP >= J

    feats16 = np.asarray(features, np.float32).astype(np.float16)
    cents16 = np.asarray(centers1, np.float32).astype(np.float16)
    targets = np.asarray(targets).astype(np.int64)

    n = np.bincount(targets, minlength=C).astype(np.int64)
    cc = n + 1

    order = np.argsort(targets, kind="stable")          # rows sorted by class
    # merged class-sorted columns: per class, feature rows then the center
    col_order = np.lexsort((
        np.concatenate([np.arange(B), np.full(C, B)]),
        np.concatenate([targets, np.arange(C)]),
    ))
    A16 = np.concatenate([feats16, cents16], axis=0)[col_order]   # [J, D]
    col_cls = np.concatenate([targets, np.arange(C)])[col_order]  # [J]

    # class sums for the numerator (f64 on the fp16 values)
    M = np.zeros((C, D))
    np.add.at(M, targets, feats16.astype(np.float64))
    M += cents16

    in_maps, host = [], []
    for k in range(n_cores):
        rids = order[k * BL:(k + 1) * BL]
        tcls = targets[rids]
        jlo = np.searchsorted(col_cls, tcls[0], "left")
        jhi = np.searchsorted(col_cls, tcls[-1], "right")
        assert jhi - jlo <= MW * P, f"core {k}: window {jhi - jlo} > {MW * P}"
        perm = np.concatenate([
            np.arange(jlo, jhi), np.arange(0, jlo), np.arange(jhi, J)
        ])
        win = perm[:MW * P]              # 10 masked tiles (window + borrow)
        restc = perm[MW * P:]            # remaining real columns

        # all positives of masked tile m must fall inside its fixed i-window
        for m in range(MW):
            wcls = col_cls[win[m * P:(m + 1) * P]]
            match = np.where(np.isin(tcls, wcls))[0]
            if len(match):
                assert WOFF[m] <= match[0] and match[-1] < WOFF[m] + WWIN, \
                    f"core {k} tile {m}: rows [{match[0]},{match[-1]}] " \
                    f"outside window {WOFF[m]}+{WWIN}"

        # sort the rest by cc; the first 4x128 become the single tiles
        # (arbitrary per-partition bias), the remainder is paired with each
        # cc bucket evened out by dummy columns
        ccr = cc[col_cls[restc]]
        o2 = np.argsort(ccr, kind="stable")
        restc, ccr = restc[o2], ccr[o2]
        sing = restc[:NSING * P]
        sing_b = np.log(1.0 / cc[col_cls[sing]]) - SHIFT
        rem, ccrem = restc[NSING * P:], ccr[NSING * P:]
        cols2, bias2, contam = [], [], 0.0   # col index (-1 = dummy), bias
        for v in np.unique(ccrem):
            idxs = rem[ccrem == v]
            b = np.log(1.0 / v) - SHIFT
            cols2.extend(idxs); bias2.extend([b] * len(idxs))
            if len(idxs) % 2:
                cols2.append(-1); bias2.append(b)
                contam += (1.0 / v) * np.exp(-SHIFT)
        npads = 2 * NPAIR * P - len(cols2)
        assert npads >= 0, f"core {k}: cc padding overflow ({-npads})"
        cols2.extend([-1] * npads); bias2.extend([-200.0] * npads)
        cols2 = np.array(cols2); bias2 = np.array(bias2)

        # j-tile layout in slot order + per-slot bias
        tiles = []
        biasm = np.empty((P, NSLOT))
        q = 0
        for s in range(NSLOT):
            if KINDS[s] == "S":
                si = SSLOTS.index(s)
                tiles.append(sing[si * P:(si + 1) * P])
                bias_col = sing_b[si * P:(si + 1) * P]
            elif KINDS[s] == "P":
                sl = slice(q * 2 * P, (q + 1) * 2 * P)
                tiles.append(cols2[sl][0::2])    # tile A: even positions
                tiles.append(cols2[sl][1::2])    # tile B: odd positions
                # bias per partition: even/odd positions share cc by design
                bias_col = bias2[sl][0::2]
                q += 1
            else:
                m = MSLOTS.index(s)
                wcols = win[m * P:(m + 1) * P]
                tiles.append(wcols)
                bias_col = np.log(1.0 / cc[col_cls[wcols]]) - SHIFT
            biasm[:, s] = bias_col
        assert q == NPAIR and len(tiles) == NT

        Ap = np.zeros((JP, D), np.float16)
        for ti, colidx in enumerate(tiles):
            real = colidx >= 0
            Ap[ti * P:(ti + 1) * P][real] = A16[colidx[real]]

        fq8 = feats16[rids].astype(F8NP)         # [BL, 128]
        A8 = Ap.astype(F8NP)                     # [JP, 128]

        im = {
            "bias": np.ascontiguousarray(biasm.astype(np.float32)),
            "tIrow": tcls.astype(np.float16).reshape(1, BL),
            "tcol": np.ascontiguousarray(
                col_cls[win].reshape(MW, P).T.astype(np.float16)
            ),
        }
        featq = np.ascontiguousarray(fq8.T)       # [128, BL]
        FS = BL // FCHUNK
        for c in range(FCHUNK):
            im[f"featq{c}"] = np.ascontiguousarray(featq[:, c * FS:(c + 1) * FS])
        fTq = A8.reshape(NT, P, P).transpose(2, 0, 1).reshape(P, NT * P)
        off = 0
        for ci, w in enumerate(WCHUNKS):
            im[f"fTq{ci}"] = np.ascontiguousarray(fTq[:, off * P:(off + w) * P])
            off += w
        in_maps.append(im)

        n_t = n[tcls].astype(np.float64)
        fqd = fq8.astype(np.float64)
        r2q = np.einsum("ij,ij->i", fqd, fqd)
        Eii = np.exp(SHIFT * r2q.astype(np.float32).astype(np.float64) - SHIFT)
        f16d = feats16[rids].astype(np.float64)
        r2n = np.einsum("ij,ij->i", f16d, f16d)
        numer = (SHIFT * (np.einsum("ij,ij->i", f16d, M[tcls]) - r2n)) / n_t - SHIFT
        host.append({"n_t": n_t, "Eii": Eii, "numer": numer, "contam": contam})
    return in_maps, host


_NC_CACHE = {}


def _get_nc():
    if "nc" not in _NC_CACHE:
        _NC_CACHE["nc"] = build_nc()
    return _NC_CACHE["nc"]


def run(centers1, features, targets, trace=False):
    n_cores = 8
    nc = _get_nc()
    in_maps, host = prep_inputs(centers1, features, targets, n_cores)
    res = run_bass_kernel_spmd(nc, in_maps, list(range(n_cores)), trace=trace)
    mlps = []
    for k in range(n_cores):
        out = res.results[k]["out"].astype(np.float64).reshape(-1)  # [2*BL]
        S = out[0:BL]
        Pm = out[BL:2 * BL]
        h = host[k]
        Sfix = S - h["contam"] + (Pm - h["Eii"]) / h["n_t"]
        mlps.append(h["numer"] - np.log(Sfix))
    loss = -np.mean(np.concatenate(mlps))
    return np.float32(loss), res


def kernel(centers1, features, targets):
    loss, _ = run(centers1, features, targets)
    return np.asarray(loss, dtype=np.float32)
